# revision 4
# baseline (speedup 1.0000x reference)
"""ChiSq (histogram_binning) Trainium2 kernel.

Math (per (batch, channel) row; F = 65537):
    xh[f] = |h[f]|^2 / psd[f]          (unscaled: the 4*DF factor is an exact
    xq[f] = |h[f]||s[f]| / psd[f]       power of two and is folded in at the end)
    si_h = cumsum(xh); targets t_k = (k/16) * total_h
    edges e_k = searchsorted(si_h, t_k, right);  G[k] = si_q[e_k]
    H[k] = G[k] - G[k-1];  chisq = (16/15) * a^2 * sum_k (H - total_q/16)^2
    with a = sqrt(0.125 / total_h);  total_out = a * total_q.

Device algorithm (data-parallel over 8 cores, 32 rows/core; each row padded
to 66048 = 4 segments x 16512, each segment = 128 blocklets x 129 elems):
  Phase A (streaming): one pass over the 5 inputs; squares/sqrt on ACT, adds
    on GPSIMD, reciprocal/multiplies/scans on DVE.  Per-segment exclusive
    prefix ph and inclusive prefix siq are written to a DRAM table of
    per-blocklet rows [ph(129) | siq(129)].  Blocklet-end prefixes (endh,
    endq) stay in SBUF.
  Phase B (tiny): cross-segment bases via 3 small PE matmuls; per-target
    blocklet location by counting endh <= t'; one indirect-DMA gather of the
    544 boundary blocklets; masked-max resolve gives G[k] exactly; final
    chisq/total assembled on 32 partitions.
"""

import sys

sys.path.insert(0, "/opt/trn_rl_repo")

import numpy as np

import concourse.bass as bass
import concourse.mybir as mybir
from concourse.bass_utils import run_bass_kernel_spmd
from concourse.tile import TileContext

AL = mybir.AluOpType
F32 = mybir.dt.float32
ACT_T = mybir.ActivationFunctionType

B, C, F = 128, 2, 65537
NCORES = 8
R = 32                      # rows per core (16 batches x 2 channels)
S = 4                       # segments per row
L = 16512                   # padded segment length = 128 * 129
ROWP = S * L                # padded row length 66048
BLK = 129                   # elements per blocklet
NBLK = 128                  # blocklets per segment
NB = 1032                   # chunk free-size (8 blocklets)
NCHUNK = L // NB            # 16
BPC = NB // BLK             # blocklets per chunk = 8
NK = 17                     # targets k = 0..16
KPAD = 20                   # padded target dim (for the [32,20]->[128,5] reshape)
W = 5                       # gather windows per partition
TROWS = R * S * NBLK        # table rows = 16384

_CACHE = {}

# ---------------------------------------------------------------------------
# Workaround: the walrus build in this container rejects instructions with
# more than one sync-wait command.  Hoist extra waits onto same-engine NOPs
# inserted right before the instruction (each engine executes its queue in
# order, so this is semantically identical).
_uid = [0]


def _split_sync_waits(nc):
    for f in nc.m.functions:
        for bb in f.blocks:
            out = []
            for inst in bb.instructions:
                si = inst.sync_info
                waits = list(si.on_wait) if (si and si.on_wait) else []
                if len(waits) > 1:
                    for wchunk in waits[:-1]:
                        _uid[0] += 1
                        out.append(
                            mybir.InstNoOp(
                                name=f"I-waitsplit-{_uid[0]}",
                                sync_info=mybir.SyncInfo(
                                    on_wait=[wchunk], on_update=[]
                                ),
                                bass_nofuse=True,
                                engine=inst.engine,
                            )
                        )
                    si.on_wait = waits[-1:]
                out.append(inst)
            bb.instructions[:] = out


class _SplitWaitTileContext(TileContext):
    def __exit__(self, *args):
        r = super().__exit__(*args)
        if args[0] is None:
            _split_sync_waits(self.nc)
        return r


# ---------------------------------------------------------------------------
def _build_nc():
    nc = bass.Bass("TRN2", target_bir_lowering=False, debug=False)

    ins = {
        n: nc.dram_tensor(n, [R, ROWP], F32, kind="ExternalInput")
        for n in ("hre", "him", "sre", "sim", "psd")
    }
    m_bcast = nc.dram_tensor("m_bcast", [128, 128], F32, kind="ExternalInput")
    m_segb = nc.dram_tensor("m_segb", [128, 128], F32, kind="ExternalInput")
    m_sel = nc.dram_tensor("m_sel", [128, 32], F32, kind="ExternalInput")
    kvec_d = nc.dram_tensor("kvec", [128, NK], F32, kind="ExternalInput")
    rv_d = nc.dram_tensor("rv512", [32, 1], F32, kind="ExternalInput")

    tbl = nc.dram_tensor("tbl", [TROWS, 2 * BLK], F32)
    o_total = nc.dram_tensor("o_total", [R, 1], F32, kind="ExternalOutput")
    o_chisq = nc.dram_tensor("o_chisq", [R, 1], F32, kind="ExternalOutput")

    # Partition convention: p = r*4 + s (row-major).  DMA APs are limited to
    # 3 dims, so the blocklet table is laid out row = p*128 + b = r*512+s*128+b.
    # input [32, 66048] -> [row, seg, elem]
    in_v = {n: t.ap().rearrange("r (s e) -> r s e", s=S) for n, t in ins.items()}
    # table [16384, 258] -> [partition, blocklet, elem]
    tbl_v = tbl.ap().rearrange("(p b) e -> p b e", p=128)

    with _SplitWaitTileContext(nc) as tc:
        with (
            tc.tile_pool(name="pin", bufs=3) as pin,
            tc.tile_pool(name="pw", bufs=2) as pw,
            tc.tile_pool(name="pscan", bufs=2) as pscan,
            tc.tile_pool(name="pers", bufs=1) as pers,
            tc.tile_pool(name="psum", bufs=1, space="PSUM") as pps,
        ):
            # constants
            c_bcast = pers.tile([128, 128], F32, tag="c_bcast")
            c_segb = pers.tile([128, 128], F32, tag="c_segb")
            c_sel = pers.tile([128, 32], F32, tag="c_sel")
            c_kvec = pers.tile([128, NK], F32, tag="c_kvec")
            c_rv = pers.tile([32, 1], F32, tag="c_rv")
            c_ones = pers.tile([128, NBLK], F32, tag="c_ones")
            nc.sync.dma_start(c_bcast[:], m_bcast.ap())
            nc.sync.dma_start(c_segb[:], m_segb.ap())
            nc.sync.dma_start(c_sel[:], m_sel.ap())
            nc.sync.dma_start(c_kvec[:], kvec_d.ap())
            nc.sync.dma_start(c_rv[:], rv_d.ap())
            nc.gpsimd.memset(c_ones[:], 1.0)

            endh = pers.tile([128, NBLK], F32, tag="endh")
            endq = pers.tile([128, NBLK], F32, tag="endq")
            stats = pers.tile([128, 53], F32, tag="stats")

            prev_xh = prev_ph = prev_siq = None
            for c in range(NCHUNK):
                base = c * NB
                t_in = {}
                for n in ("hre", "him", "sre", "sim", "psd"):
                    t = pin.tile([128, NB], F32, tag=n)
                    nc.sync.dma_start(t[:], in_v[n][:, :, base : base + NB])
                    t_in[n] = t
                # squares in place (ACT)
                for n in ("hre", "him", "sre", "sim"):
                    nc.scalar.square(t_in[n][:], t_in[n][:])
                ah = pw.tile([128, NB], F32, tag="ah")
                as_ = pw.tile([128, NB], F32, tag="as")
                nc.gpsimd.tensor_tensor(ah[:], t_in["hre"][:], t_in["him"][:], AL.add)
                nc.gpsimd.tensor_tensor(as_[:], t_in["sre"][:], t_in["sim"][:], AL.add)
                pinv = pw.tile([128, NB], F32, tag="pinv")
                nc.vector.reciprocal(pinv[:], t_in["psd"][:])
                xh = pscan.tile([128, NB + 1], F32, tag="xh")
                nc.vector.tensor_tensor(xh[:, 1 : NB + 1], ah[:], pinv[:], AL.mult)
                if c == 0:
                    nc.gpsimd.memset(xh[:, 0:1], 0.0)
                else:
                    nc.scalar.copy(xh[:, 0:1], prev_xh[:, NB : NB + 1])
                pr = pw.tile([128, NB], F32, tag="pr")
                nc.vector.tensor_tensor(pr[:], ah[:], as_[:], AL.mult)
                nc.scalar.sqrt(pr[:], pr[:])
                xq = pw.tile([128, NB], F32, tag="xq")
                nc.vector.tensor_tensor(xq[:], pr[:], pinv[:], AL.mult)

                ph = pscan.tile([128, NB], F32, tag="ph")
                init_h = 0.0 if c == 0 else prev_ph[:, NB - 1 : NB]
                nc.vector.tensor_tensor_scan(
                    ph[:], xh[:, 0:NB], xh[:, 0:NB], init_h, AL.add, AL.bypass
                )
                siq = pscan.tile([128, NB], F32, tag="siq")
                init_q = 0.0 if c == 0 else prev_siq[:, NB - 1 : NB]
                nc.vector.tensor_tensor_scan(
                    siq[:], xq[:], xq[:], init_q, AL.add, AL.bypass
                )

                # blocklet-end prefixes: inclusive end = ph[next] is not
                # available at the last blocklet, so use ph(end) + xh(end)
                cb = c * BPC
                nc.vector.tensor_tensor(
                    endh[:, cb : cb + BPC],
                    ph[:, BLK - 1 : NB : BLK],
                    xh[:, BLK : NB + 1 : BLK],
                    AL.add,
                )
                nc.scalar.copy(endq[:, cb : cb + BPC], siq[:, BLK - 1 : NB : BLK])

                # stream the prefix arrays to the DRAM blocklet table
                nc.sync.dma_start(
                    tbl_v[:, cb : cb + BPC, 0:BLK],
                    ph[:].rearrange("p (b e) -> p b e", b=BPC),
                )
                nc.sync.dma_start(
                    tbl_v[:, cb : cb + BPC, BLK : 2 * BLK],
                    siq[:].rearrange("p (b e) -> p b e", b=BPC),
                )
                prev_xh, prev_ph, prev_siq = xh, ph, siq

            # ---- phase B ----
            # segment totals
            nc.vector.tensor_tensor(
                stats[:, 51:52], prev_ph[:, NB - 1 : NB], prev_xh[:, NB : NB + 1], AL.add
            )
            nc.scalar.copy(stats[:, 52:53], prev_siq[:, NB - 1 : NB])

            ps_row = pps.tile([128, 2], F32, tag="ps_row")
            ps_base = pps.tile([128, 2], F32, tag="ps_base")
            nc.tensor.matmul(
                out=ps_row[:], lhsT=c_bcast[:], rhs=stats[:, 51:53], start=True, stop=True
            )
            nc.tensor.matmul(
                out=ps_base[:], lhsT=c_segb[:], rhs=stats[:, 51:53], start=True, stop=True
            )
            rowtot = pers.tile([128, 2], F32, tag="rowtot")   # TH_row, TQ_row bcast
            segbase = pers.tile([128, 2], F32, tag="segbase")  # segbase_h, segbase_q
            nc.vector.tensor_copy(rowtot[:], ps_row[:])
            nc.vector.tensor_copy(segbase[:], ps_base[:])

            # t'[p,k] = kvec*TH_row - segbase_h
            tprime = pers.tile([128, NK], F32, tag="tprime")
            nc.vector.tensor_scalar(
                tprime[:], c_kvec[:], rowtot[:, 0:1], segbase[:, 0:1],
                AL.mult, AL.subtract,
            )
            # counting: cnt[p,k] = #(endh <= t'_k)  (accumulated mask sum)
            msk = pw.tile([128, NBLK], F32, tag="msk")
            for k in range(NK):
                msk = pw.tile([128, NBLK], F32, tag="msk")
                nc.vector.scalar_tensor_tensor(
                    msk[:], endh[:], tprime[:, k : k + 1], c_ones[:],
                    AL.is_le, AL.mult, accum_out=stats[:, k : k + 1],
                )
            # full[p,k] = (cnt == 128);  prodh/prodq = full * seg totals
            full = pers.tile([128, NK], F32, tag="full")
            nc.vector.tensor_scalar(full[:], stats[:, 0:NK], float(NBLK), None, AL.is_equal)
            nc.vector.tensor_scalar(
                stats[:, 17:34], full[:], stats[:, 51:52], None, AL.mult
            )
            nc.vector.tensor_scalar(
                stats[:, 34:51], full[:], stats[:, 52:53], None, AL.mult
            )
            ps_st = pps.tile([32, 53], F32, tag="ps_st")
            nc.tensor.matmul(
                out=ps_st[:], lhsT=c_sel[:], rhs=stats[:], start=True, stop=True
            )
            st32 = pers.tile([32, 53], F32, tag="st32")
            nc.vector.tensor_copy(st32[:], ps_st[:])
            # st32: m=[:,0:17] sigh=[:,17:34] sigq=[:,34:51] THr=[:,51:52] TQr=[:,52:53]

            # tau = t_k(row) - sigma_h ; sigma_q per pair ; rowidx
            tau = pers.tile([32, KPAD], F32, tag="tau")
            nc.vector.tensor_scalar(
                tau[:, 0:NK], c_kvec[0:32, :], st32[:, 51:52], None, AL.mult
            )
            nc.vector.tensor_tensor(
                tau[:, 0:NK], tau[:, 0:NK], st32[:, 17:34], AL.subtract
            )
            nc.vector.tensor_copy(
                tau[:, NK:KPAD], tau[:, 16:17].to_broadcast([32, KPAD - NK])
            )
            sq20 = pers.tile([32, KPAD], F32, tag="sq20")
            nc.vector.tensor_copy(sq20[:, 0:NK], st32[:, 34:51])
            nc.vector.tensor_copy(
                sq20[:, NK:KPAD], st32[:, 50:51].to_broadcast([32, KPAD - NK])
            )
            ridxf = pers.tile([32, KPAD], F32, tag="ridxf")
            nc.vector.tensor_scalar(
                ridxf[:, 0:NK], st32[:, 0:NK], float(S * NBLK - 1), c_rv[:],
                AL.min, AL.add,
            )
            nc.vector.tensor_copy(
                ridxf[:, NK:KPAD], ridxf[:, 16:17].to_broadcast([32, KPAD - NK])
            )
            ridxi = pers.tile([32, KPAD], mybir.dt.int32, tag="ridxi")
            nc.vector.tensor_copy(ridxi[:], ridxf[:])

            # [32,20] -> [128,5] reshapes
            tau128 = pers.tile([128, W], F32, tag="tau128")
            sq128 = pers.tile([128, W], F32, tag="sq128")
            ridx128 = pers.tile([128, W], mybir.dt.int32, tag="ridx128")
            nc.sync.dma_start(tau128[:], tau[:].rearrange("r (g w) -> r g w", g=4))
            nc.sync.dma_start(sq128[:], sq20[:].rearrange("r (g w) -> r g w", g=4))
            nc.sync.dma_start(ridx128[:], ridxi[:].rearrange("r (g w) -> r g w", g=4))

            # gather boundary blocklets: row (p,w) <- tbl[ridx128[p,w], :]
            gath = pers.tile([128, W * 2 * BLK], F32, tag="gath")
            for w in range(W):
                nc.gpsimd.indirect_dma_start(
                    out=gath[:, w * 2 * BLK : (w + 1) * 2 * BLK],
                    out_offset=None,
                    in_=tbl.ap(),
                    in_offset=bass.IndirectOffsetOnAxis(
                        ap=ridx128[:, w : w + 1], axis=0
                    ),
                )
            # resolve: G = max(siq_win * (ph_win <= tau)) + sigma_q
            gmax = pers.tile([128, W], F32, tag="gmax")
            for w in range(W):
                prodw = pw.tile([128, BLK], F32, tag="prodw")
                o = w * 2 * BLK
                nc.vector.scalar_tensor_tensor(
                    prodw[:], gath[:, o : o + BLK], tau128[:, w : w + 1],
                    gath[:, o + BLK : o + 2 * BLK], AL.is_le, AL.mult,
                )
                nc.vector.tensor_reduce(
                    gmax[:, w : w + 1], prodw[:], mybir.AxisListType.X, AL.max
                )
            nc.vector.tensor_tensor(gmax[:], gmax[:], sq128[:], AL.add)

            g32 = pers.tile([32, KPAD], F32, tag="g32")
            nc.sync.dma_start(g32[:].rearrange("r (g w) -> r g w", g=4), gmax[:])
            # exact endpoint: G[16] = TQ_row
            nc.scalar.copy(g32[:, 16:17], st32[:, 52:53])

            # finals
            hs = pers.tile([32, 16], F32, tag="hs")
            nc.vector.tensor_tensor(hs[:], g32[:, 1:NK], g32[:, 0:16], AL.subtract)
            mu = pers.tile([32, 1], F32, tag="mu")
            nc.vector.tensor_scalar(mu[:], st32[:, 52:53], 1.0 / 16.0, None, AL.mult)
            nc.vector.tensor_scalar(hs[:], hs[:], mu[:], None, AL.subtract)
            sqd = pers.tile([32, 16], F32, tag="sqd")
            ssq = pers.tile([32, 1], F32, tag="ssq")
            nc.scalar.activation(sqd[:], hs[:], ACT_T.Square, accum_out=ssq[:])
            rth = pers.tile([32, 1], F32, tag="rth")
            nc.vector.reciprocal(rth[:], st32[:, 51:52])
            chq = pers.tile([32, 1], F32, tag="chq")
            nc.vector.tensor_scalar(chq[:], ssq[:], rth[:], 2.0 / 15.0, AL.mult, AL.mult)
            alpha = pers.tile([32, 1], F32, tag="alpha")
            nc.vector.tensor_scalar(alpha[:], rth[:], 0.125, None, AL.mult)
            nc.scalar.sqrt(alpha[:], alpha[:])
            toto = pers.tile([32, 1], F32, tag="toto")
            nc.vector.tensor_tensor(toto[:], st32[:, 52:53], alpha[:], AL.mult)

            nc.sync.dma_start(o_total.ap(), toto[:])
            nc.sync.dma_start(o_chisq.ap(), chq[:])

    return nc


def _consts():
    # partition p = r*4 + s
    kp = np.arange(128)
    m_bcast = (kp[:, None] // 4 == kp[None, :] // 4).astype(np.float32)
    m_segb = (
        (kp[:, None] // 4 == kp[None, :] // 4) & (kp[:, None] % 4 < kp[None, :] % 4)
    ).astype(np.float32)
    m_sel = (kp[:, None] // 4 == np.arange(32)[None, :]).astype(np.float32)
    kvec = np.broadcast_to(np.arange(NK, dtype=np.float32) / 16.0, (128, NK)).copy()
    rv = (np.arange(32, dtype=np.float32) * (S * NBLK))[:, None].copy()
    return {
        "m_bcast": m_bcast,
        "m_segb": m_segb,
        "m_sel": m_sel,
        "kvec": kvec,
        "rv512": rv,
    }


def kernel(htilde_re, htilde_im, stilde_re, stilde_im, psd):
    if "nc" not in _CACHE:
        _CACHE["nc"] = _build_nc()
        _CACHE["consts"] = _consts()
    nc = _CACHE["nc"]
    consts = _CACHE["consts"]

    arrs = {
        "hre": np.asarray(htilde_re, np.float32),
        "him": np.asarray(htilde_im, np.float32),
        "sre": np.asarray(stilde_re, np.float32),
        "sim": np.asarray(stilde_im, np.float32),
        "psd": np.asarray(psd, np.float32),
    }
    bpc = B // NCORES  # batches per core
    in_maps = []
    for c in range(NCORES):
        m = dict(consts)
        for n, a in arrs.items():
            rows = a[c * bpc : (c + 1) * bpc].reshape(R, F)
            pad = np.zeros((R, ROWP), np.float32) if n != "psd" else np.ones(
                (R, ROWP), np.float32
            )
            pad[:, :F] = rows
            m[n] = pad
        in_maps.append(m)

    res = run_bass_kernel_spmd(nc, in_maps, core_ids=list(range(NCORES)))
    total = np.empty((B, C), np.float32)
    chisq = np.empty((B, C), np.float32)
    for c in range(NCORES):
        total[c * bpc : (c + 1) * bpc] = res.results[c]["o_total"].reshape(bpc, C)
        chisq[c * bpc : (c + 1) * bpc] = res.results[c]["o_chisq"].reshape(bpc, C)
    return total, chisq
